# revision 40
# baseline (speedup 1.0000x reference)
"""Fused causal attention head (QKV proj + causal softmax attention) on 8 trn2 cores.

Sharding: core = 4*b + r (b = batch of 2, r = rank in a 4-core group).
  - Queries: core handles row chunks [512r, 512r+512) and [512(7-r), 512(8-r))
    of its batch (pairing r with 7-r balances causal attention work).
  - K/V: core projects keys [1024r, 1024(r+1)); shards are exchanged with 4
    pipelined 8-core AllGathers (K and V per key *half*, Shared outputs ->
    RDH fast path: ~40us per 1MB-in AG vs ~26us per 512KB-in 4-core mesh
    AG). Each core consumes only its batch's half of each AG output via a
    dynamic-offset DMA (offset = (core>=4) * half); explicit dep edges pin
    those loads behind the collective.
  - A tiny 8-core AllGather is emitted first so the ~40us NRT comm-init
    barrier + stream priming overlap the input DMAs / projections.
  - PE order: warmup, K01 proj, Q^T, V01 proj, K23 proj, V23 proj, then
    attention; Q^T is early so scores can start as soon as AG K01 lands.
Attention is computed in a transposed-scores layout (keys on PSUM partitions):
  S^T = K^T-chunk^T-matmul-Q^T, P^T = exp(S^T/32) * causal01mask,
  partial O/rowsum accumulate in PSUM per (quarter, query-group), then fold
  into SBUF fp32 accumulators; final O /= rowsum.
  No max-subtraction: scores are ~N(0,1) so exp cannot overflow fp32.
"""

import os
import sys

sys.path.insert(0, "/opt/trn_rl_repo")

import numpy as np
import ml_dtypes

B, S, D = 2, 4096, 1024
NCORES = 8
P = 128
NQ = 1024          # queries per core
QG = 256           # query group (scores matmul free dim)
NQG = NQ // QG     # 4
KB = 512
QK = 256           # quarter-of-rank key block
BF16 = ml_dtypes.bfloat16

# queries are stride-4 interleaved: core 4b+r handles batch-b positions
# {4i + r}. qg g (local i in [256g, 256g+256)) spans positions
# [1024g, 1024(g+1)) -> needs key ranks 0..g only.
RRS = ((0,), (0, 1), (0, 1, 2), (0, 1, 2, 3))
# score tiles per query-group PAIR: (rank, column offset, width); ranks
# needed by only the pair's second qg get a half-width tile
SCORE_TILES = (
    ((0, 0, 512), (1, 256, 256)),
    ((0, 0, 512), (1, 0, 512), (2, 0, 512), (3, 256, 256)),
)
NSTEP = 4 * sum(len(t) for t in SCORE_TILES)  # 24 mask tiles of [P, 2, 512]

LAST_EXEC_NS = None
WARMUP = int(os.environ.get("KWARMUP", "24"))

_built = {}


def _build():
    import concourse.bacc as bacc
    import concourse.tile as tile
    import concourse.mybir as mybir
    from concourse.bass_types import AP
    from concourse.tile import add_dep_helper
    from concourse.masks import make_identity

    nc = bacc.Bacc("TRN2", target_bir_lowering=False, debug=False,
                   num_devices=NCORES)
    dt = mybir.dt

    xq_t = nc.dram_tensor("xq_t", [D, NQ], dt.bfloat16, kind="ExternalInput").ap()
    xkv_t = nc.dram_tensor("xkv_t", [D, 1024], dt.bfloat16, kind="ExternalInput").ap()
    w = nc.dram_tensor("w", [D, 3 * D], dt.bfloat16, kind="ExternalInput").ap()
    maskt = nc.dram_tensor("maskt", [NSTEP, P, 2, 2 * QG], dt.bfloat16,
                           kind="ExternalInput").ap()
    out = nc.dram_tensor("out", [NQ, D], dt.float32, kind="ExternalOutput").ap()

    DC = D // P  # 8 contraction chunks
    RG8 = [[0, 1, 2, 3, 4, 5, 6, 7]]

    with tile.TileContext(nc, num_cores=NCORES) as tc:
        with (
            tc.tile_pool(name="persist", bufs=1) as persist,
            tc.tile_pool(name="dram", bufs=1, space="DRAM") as dram,
        ):
            qt_sb = persist.tile([P, DC, NQ], dt.bfloat16)
            ones_sb = persist.tile([P, P], dt.bfloat16)
            ident = persist.tile([P, P], dt.float32)

            # ---- tiny 8-core AG first: absorbs the NRT comm-init barrier
            # + CC stream priming under the input DMAs / projections.
            dum_sb = persist.tile([1, 16], dt.bfloat16)
            nc.vector.memset(dum_sb, 0.0)
            dum_in = dram.tile([1, 16], dt.bfloat16)
            dum_out = dram.tile([8, 16], dt.bfloat16)
            nc.scalar.dma_start(dum_in, dum_sb)
            nc.gpsimd.collective_compute(
                "AllGather", mybir.AluOpType.bypass, replica_groups=RG8,
                ins=[dum_in.opt()], outs=[dum_out.opt()])

            nc.vector.memset(ones_sb, 1.0)
            make_identity(nc, ident)

            # K halves: [1024 d-rows, 512 keys] per core -> out [8192, 512]
            agin_k = [dram.tile([D, 2 * QK], dt.bfloat16, name=f"agin_k{h}")
                      for h in range(2)]
            agout_k = [dram.tile([8 * D, 2 * QK], dt.bfloat16,
                                 name=f"agout_k{h}", addr_space="Shared")
                       for h in range(2)]
            # V halves: [512 keys, 1024 d] per core -> out [4096, 1024]
            agin_v = [dram.tile([2 * QK, D], dt.bfloat16, name=f"agin_v{h}")
                      for h in range(2)]
            agout_v = [dram.tile([8 * 2 * QK, D], dt.bfloat16,
                                 name=f"agout_v{h}", addr_space="Shared")
                       for h in range(2)]
            cc_k = [None, None]
            cc_v = [None, None]

            # ---- Phase 1: projections + 4 pipelined 8-core AllGathers ----
            with (
                tc.tile_pool(name="projbuf", bufs=1) as projbuf,
                tc.tile_pool(name="projtmp", bufs=4) as projtmp,
                tc.tile_pool(name="projps", bufs=4, space="PSUM") as projps,
            ):
                # PE warmup while input DMAs stream
                if WARMUP:
                    wu = projbuf.tile([P, KB], dt.bfloat16)
                    nc.vector.memset(wu, 0.0)
                    wu_ps = projps.tile([P, KB], dt.float32, tag="pps",
                                        name="wu_ps")
                    for i in range(WARMUP):
                        nc.tensor.matmul(wu_ps, lhsT=wu[:, :P], rhs=wu,
                                         start=True, stop=True)

                # alloc order matters: xq/w_q are released right after Q^T
                # (~80us); placing them last puts them at the SBUF boundary
                # where phase-2 K/V tiles overlap, so those loads unblock as
                # soon as Q^T is done instead of after all projections
                w_k = projbuf.tile([P, DC, D], dt.bfloat16)
                w_v = projbuf.tile([P, DC, D], dt.bfloat16)
                xkv_sb = projbuf.tile([P, DC, 1024], dt.bfloat16)
                xq_sb = projbuf.tile([P, DC, NQ], dt.bfloat16)
                w_q = projbuf.tile([P, DC, D], dt.bfloat16)
                w_r = w.rearrange("(c p) n -> p c n", p=P)
                # split across the two HWDGE queues so K-proj inputs land
                # first; keep gpsimd free of everything but the collective
                # triggers (smaller ucode library -> earlier first trigger)
                nc.sync.dma_start(xkv_sb, xkv_t.rearrange("(c p) n -> p c n", p=P))
                nc.scalar.dma_start(w_k, w_r[:, :, D:2 * D])
                nc.sync.dma_start(w_q, w_r[:, :, 0:D])
                nc.scalar.dma_start(xq_sb, xq_t.rearrange("(c p) n -> p c n", p=P))
                nc.sync.dma_start(w_v, w_r[:, :, 2 * D:3 * D])

                def proj_k_half(h):
                    agin_k_r = agin_k[h].rearrange("(m p) k -> m p k", p=P)
                    for m in range(DC):
                        kt_ps = projps.tile([P, 2 * QK], dt.float32,
                                            tag="ppsk", name="kt_ps")
                        for c in range(DC):
                            nc.tensor.matmul(
                                kt_ps,
                                lhsT=w_k[:, c, m * P:(m + 1) * P],
                                rhs=xkv_sb[:, c, h * 2 * QK:(h + 1) * 2 * QK],
                                start=(c == 0), stop=(c == DC - 1),
                            )
                        kt_bf = projtmp.tile([P, 2 * QK], dt.bfloat16,
                                             tag="pck")
                        nc.vector.tensor_copy(kt_bf, kt_ps)
                        nc.scalar.dma_start(agin_k_r[m], kt_bf)

                def ag_k_half(h):
                    cc_k[h] = nc.gpsimd.collective_compute(
                        "AllGather", mybir.AluOpType.bypass, replica_groups=RG8,
                        ins=[agin_k[h].opt()], outs=[agout_k[h].opt()])

                def proj_v_quarter(q):
                    h, qh = divmod(q, 2)
                    agin_v_r = agin_v[h].rearrange("(m p) d -> m p d", p=P)
                    for m in range(2):
                        for nh in range(2):
                            v_ps = projps.tile([P, KB], dt.float32, tag="pps",
                                               name="v_ps")
                            for c in range(DC):
                                nc.tensor.matmul(
                                    v_ps,
                                    lhsT=xkv_sb[:, c,
                                                q * QK + m * P:
                                                q * QK + (m + 1) * P],
                                    rhs=w_v[:, c, nh * KB:(nh + 1) * KB],
                                    start=(c == 0), stop=(c == DC - 1),
                                )
                            v_bf = projtmp.tile([P, KB], dt.bfloat16,
                                                tag="pcopy")
                            nc.vector.tensor_copy(v_bf, v_ps)
                            nc.scalar.dma_start(
                                agin_v_r[qh * 2 + m][:, nh * KB:(nh + 1) * KB],
                                v_bf)

                def ag_v_half(h):
                    cc_v[h] = nc.gpsimd.collective_compute(
                        "AllGather", mybir.AluOpType.bypass, replica_groups=RG8,
                        ins=[agin_v[h].opt()], outs=[agout_v[h].opt()])

                def proj_q():
                    for m in range(DC):
                        for nh in range(2):
                            q_ps = projps.tile([P, KB], dt.float32, tag="pps",
                                               name="q_ps")
                            for c in range(DC):
                                nc.tensor.matmul(
                                    q_ps,
                                    lhsT=w_q[:, c, m * P:(m + 1) * P],
                                    rhs=xq_sb[:, c, nh * KB:(nh + 1) * KB],
                                    start=(c == 0), stop=(c == DC - 1),
                                )
                            nc.vector.tensor_copy(
                                qt_sb[:, m, nh * KB:(nh + 1) * KB], q_ps)

                # Wire order K01 K23 V01 V23: K fronted so all scores can
                # run early; V halves land just-in-time for the PV phases,
                # leaving only pv_half(1) after the last AG.
                proj_k_half(0)
                ag_k_half(0)
                proj_k_half(1)
                ag_k_half(1)
                proj_q()
                proj_v_quarter(0)
                proj_v_quarter(1)
                ag_v_half(0)
                proj_v_quarter(2)
                proj_v_quarter(3)
                ag_v_half(1)

            # ---- Phase 2: attention, quarter-major ----
            _phase2(nc, tc, mybir, AP, add_dep_helper, qt_sb, ones_sb, ident,
                    agout_k, agout_v, cc_k, cc_v, maskt, out)

    nc.compile()
    return nc


def _dyn_batch_ap(AP, base_ap, sel, half_elems):
    """base_ap shifted by sel*half_elems elements (sel = 0/1 runtime value)."""
    return AP(
        tensor=base_ap.tensor,
        offset=base_ap.offset + sel * half_elems,
        ap=base_ap.ap,
        dep_tracking_offset=base_ap.offset,
    )


def _phase2(nc, tc, mybir, AP, add_dep_helper, qt_sb, ones_sb, ident,
            agout_k, agout_v, cc_k, cc_v, maskt, out):
    dt = mybir.dt
    DC = D // P
    # batch selector (0 for cores 0-3, 1 for cores 4-7) per DMA engine
    sel_sync = nc.sync.partition_id() >= 4
    sel_scal = nc.scalar.partition_id() >= 4
    k_half_elems = 4 * D * 2 * QK      # 4096 rows x 512 cols
    v_half_elems = 4 * 2 * QK * D      # 2048 rows x 1024 cols
    with (
        tc.tile_pool(name="acc", bufs=1) as accpool,
        tc.tile_pool(name="kvq", bufs=1) as kvqpool,
        tc.tile_pool(name="mask", bufs=8) as maskpool,
        tc.tile_pool(name="pt", bufs=3) as ptpool,
        tc.tile_pool(name="norm", bufs=2) as normpool,
        tc.tile_pool(name="osb", bufs=2) as osbpool,
        tc.tile_pool(name="ops", bufs=5, space="PSUM") as opspool,
        tc.tile_pool(name="stps", bufs=2, space="PSUM") as stpspool,
        tc.tile_pool(name="sumps", bufs=1, space="PSUM") as sumpspool,
    ):
        o_acc = [[accpool.tile([P, D], dt.float32, name=f"oacc{qg}_{qs}")
                  for qs in range(2)] for qg in range(NQG)]
        sum_acc = [accpool.tile([P, 2 * QG], dt.float32, name=f"sacc{qgp}")
                   for qgp in range(2)]

        kall = {}      # h -> K^T tile [P, 32, 512]  (index rr*DC + c)
        vall = {}      # h -> V tile [P, 16, 1024]   (index rr*4 + qh*2 + kt)
        pt_tiles = {}  # (q, qgp, rr, kt) -> P^T tile
        state = {"step": 0}

        def load_k(h):
            kt_t = kvqpool.tile([P, 4 * DC, 2 * QK], dt.bfloat16,
                                tag="kall", name=f"kall{h}")
            src = agout_k[h][0:4 * D].rearrange("(a p) k -> p a k", p=P)
            # two parallel 2MB loads (ranks 0-1 on sync, 2-3 on scalar):
            # scores' first tiles need rank 0/1 soonest
            ld = nc.sync.dma_start(
                kt_t[:, 0:2 * DC],
                _dyn_batch_ap(AP, src[:, 0:2 * DC], sel_sync, k_half_elems))
            add_dep_helper(ld.ins, cc_k[h].ins,
                           reason="k load waits on AG completion")
            ld = nc.scalar.dma_start(
                kt_t[:, 2 * DC:4 * DC],
                _dyn_batch_ap(AP, src[:, 2 * DC:4 * DC], sel_scal,
                              k_half_elems))
            add_dep_helper(ld.ins, cc_k[h].ins,
                           reason="k load waits on AG completion")
            kall[h] = kt_t

        def load_v(h):
            v_t = kvqpool.tile([P, 16, D], dt.bfloat16, tag="vall",
                               name=f"vall{h}")
            src = agout_v[h][0:8 * QK].rearrange("(a p) d -> p a d", p=P)
            ld = nc.sync.dma_start(
                v_t[:, 0:8],
                _dyn_batch_ap(AP, src[:, 0:8], sel_sync, v_half_elems))
            add_dep_helper(ld.ins, cc_v[h].ins,
                           reason="v load waits on AG completion")
            ld = nc.scalar.dma_start(
                v_t[:, 8:16],
                _dyn_batch_ap(AP, src[:, 8:16], sel_scal, v_half_elems))
            add_dep_helper(ld.ins, cc_v[h].ins,
                           reason="v load waits on AG completion")
            vall[h] = v_t

        def pass_scores(q):
            h, qh = divmod(q, 2)
            for qgp in range(2):
                qoff = qgp * 2 * QG
                tiles = SCORE_TILES[qgp]
                sum_ps = sumpspool.tile([P, 2 * QG], dt.float32,
                                        tag="sum_ps")
                for rr, off, wd in tiles:
                    mask_sb = maskpool.tile([P, 2, 2 * QG], dt.bfloat16,
                                            tag="mask")
                    nc.sync.dma_start(mask_sb, maskt[state["step"]])
                    for kt in range(2):
                        st_ps = stpspool.tile([P, 2 * QG], dt.float32,
                                              tag="st")
                        koff = qh * QK + kt * P
                        for c in range(DC):
                            nc.tensor.matmul(
                                st_ps[:, 0:wd],
                                lhsT=kall[h][:, rr * DC + c, koff:koff + P],
                                rhs=qt_sb[:, c, qoff + off:qoff + off + wd],
                                start=(c == 0), stop=(c == DC - 1),
                            )
                        pt_sb = ptpool.tile([P, 2 * QG], dt.bfloat16,
                                            tag="pt", bufs=50,
                                            name=f"pt{q}_{qgp}_{rr}_{kt}")
                        nc.scalar.activation(
                            out=pt_sb[:, 0:wd], in_=st_ps[:, 0:wd],
                            func=mybir.ActivationFunctionType.Exp,
                            scale=float(1.0 / np.sqrt(D)),
                        )
                        nc.vector.tensor_mul(
                            pt_sb[:, 0:wd], pt_sb[:, 0:wd],
                            mask_sb[:, kt, off:off + wd])
                        pt_tiles[(q, qgp, rr, kt)] = (pt_sb, off, wd)
                    state["step"] += 1
                # rowsums after the pair's score tiles: no PE bubble waiting
                # for each tile's exp+mask to drain
                for i, (rr, off, wd) in enumerate(tiles):
                    for kt in range(2):
                        nc.tensor.matmul(
                            sum_ps[:, off:off + wd], lhsT=ones_sb,
                            rhs=pt_tiles[(q, qgp, rr, kt)][0][:, 0:wd],
                            start=(i == 0 and kt == 0),
                            stop=(i == len(tiles) - 1 and kt == 1),
                        )
                # fold the pair's rowsum into the SBUF accumulator
                if q == 0:
                    nc.vector.tensor_copy(sum_acc[qgp], sum_ps)
                else:
                    nc.vector.tensor_add(sum_acc[qgp], sum_acc[qgp], sum_ps)

        def pass_pv_half(h):
            # both quarters of half h accumulate into one PSUM group per qg:
            # half as many PSUM->SBUF folds
            for qg in range(NQG):
                qgp, qglo = divmod(qg, 2)
                rrs = RRS[qg]
                po = qglo * QG
                # four 1-bank partial-O tiles (qs, dn) with 5 slots so the
                # next qg can start accumulating while folds drain
                o_ps = [opspool.tile([P, KB], dt.float32, tag="opart", bufs=5,
                                     name=f"o_{h}_{qg}_{i}")
                        for i in range(4)]
                for q in (2 * h, 2 * h + 1):
                    qh = q % 2
                    for rr in rrs:
                        for kt in range(2):
                            ent = (pt_tiles.pop((q, qgp, rr, kt))
                                   if qglo == 1
                                   else pt_tiles[(q, qgp, rr, kt)])
                            pt_sb, off, wd = ent
                            mm_start = qh == 0 and rr == rrs[0] and kt == 0
                            mm_stop = qh == 1 and rr == rrs[-1] and kt == 1
                            for qs in range(2):
                                for dn in range(2):
                                    nc.tensor.matmul(
                                        o_ps[qs * 2 + dn],
                                        lhsT=pt_sb[:, po - off + qs * P:
                                                   po - off + (qs + 1) * P],
                                        rhs=vall[h][:, rr * 4 + qh * 2 + kt,
                                                    dn * KB:(dn + 1) * KB],
                                        start=mm_start, stop=mm_stop,
                                    )
                # fold partials into SBUF accumulators
                for qs in range(2):
                    for dn in range(2):
                        dst = o_acc[qg][qs][:, dn * KB:(dn + 1) * KB]
                        if h == 0:
                            nc.vector.tensor_copy(dst, o_ps[qs * 2 + dn])
                        else:
                            nc.vector.tensor_add(dst, dst, o_ps[qs * 2 + dn])

        # emission order matches the AG wire order K01 K23 V01 V23
        load_k(0)
        pass_scores(0)
        pass_scores(1)
        load_k(1)
        pass_scores(2)
        pass_scores(3)
        load_v(0)
        pass_pv_half(0)
        load_v(1)
        pass_pv_half(1)

        assert state["step"] == NSTEP

        # ---- normalize: O /= rowsum ----
        for qg in range(NQG):
            qoff = qg * QG
            qgp, qglo = divmod(qg, 2)
            for qs in range(2):
                o_sb = osbpool.tile([P, D], dt.float32, tag="o_sb")
                sumt_ps = stpspool.tile([P, P], dt.float32, tag="st")
                nc.tensor.transpose(
                    sumt_ps,
                    sum_acc[qgp][:, qglo * QG + qs * P:
                                 qglo * QG + (qs + 1) * P], ident)
                recip = normpool.tile([P, 1], dt.float32, tag="recip")
                nc.vector.reciprocal(recip, sumt_ps[:, 0:1])
                nc.vector.tensor_scalar_mul(o_sb, o_acc[qg][qs], recip)
                nc.scalar.dma_start(
                    out[qoff + qs * P:qoff + (qs + 1) * P, :], o_sb)


def _get_nc():
    if "nc" not in _built:
        _built["nc"] = _build()
    return _built["nc"]


def _host_inputs(x, W):
    """Build the 8 per-core input maps from the full inputs."""
    x = np.asarray(x)
    W = np.asarray(W)
    w_bf = W.astype(BF16)

    in_maps = []
    for core in range(NCORES):
        b, r = divmod(core, 4)
        xq = x[b, r::4]                                            # [1024, D]
        xkv = x[b, 1024 * r:1024 * (r + 1)]                        # [1024, D]
        in_maps.append({
            "xq_t": np.ascontiguousarray(xq.T).astype(BF16),
            "xkv_t": np.ascontiguousarray(xkv.T).astype(BF16),
            "w": w_bf,
            "maskt": _masks_for_rank(r),
        })
    return in_maps


_mask_cache = {}


def _masks_for_rank(r):
    if r in _mask_cache:
        return _mask_cache[r]
    qpos = 4 * np.arange(NQ, dtype=np.int64) + r
    m = np.zeros((NSTEP, P, 2, 2 * QG), dtype=BF16)
    step = 0
    for q in range(4):
        for qgp in range(2):
            qp = qpos[qgp * 2 * QG:(qgp + 1) * 2 * QG]
            for rr, off, wd in SCORE_TILES[qgp]:
                for kt in range(2):
                    kpos = 1024 * rr + QK * q + kt * P + np.arange(P)
                    m[step, :, kt, :] = (
                        kpos[:, None] <= qp[None, :]).astype(BF16)
                step += 1
    assert step == NSTEP
    _mask_cache[r] = m
    return m


def _gather(results):
    out = np.empty((B, S, D), dtype=np.float32)
    for core in range(NCORES):
        b, r = divmod(core, 4)
        out[b, r::4] = results[core]["out"]
    return out


def kernel(x, W):
    global LAST_EXEC_NS
    from concourse import bass_utils

    nc = _get_nc()
    in_maps = _host_inputs(x, W)
    trace = os.environ.get("BASS_KERNEL_TRACE", "0") == "1"
    if trace:
        try:
            import types
            import antenv
            if "antenv.axon_hooks" not in sys.modules:
                mod = types.ModuleType("antenv.axon_hooks")
                _hook = [None]
                mod.set_axon_ntff_profile_hook = (
                    lambda fn: _hook.__setitem__(0, fn))
                mod.get_axon_ntff_profile_hook = lambda: _hook[0]
                sys.modules["antenv.axon_hooks"] = mod
                antenv.axon_hooks = mod
            from antenv.axon_hooks import (
                get_axon_ntff_profile_hook, set_axon_ntff_profile_hook)
            if get_axon_ntff_profile_hook() is None:
                from trn_agent_boot.trn_boot import _ntff_profile_via_ctypes
                set_axon_ntff_profile_hook(
                    _ntff_profile_via_ctypes("/opt/axon/libaxon_pjrt.so"))
        except Exception:
            pass
    res = bass_utils.run_bass_kernel_spmd(
        nc, in_maps, core_ids=list(range(NCORES)), trace=trace,
        tmpdir=os.environ.get("BASS_KERNEL_TRACE_DIR") or None,
    )
    LAST_EXEC_NS = res.exec_time_ns
    return _gather(res.results)


# revision 43
# speedup vs baseline: 1.0241x; 1.0241x over previous
"""Fused causal attention head (QKV proj + causal softmax attention) on 8 trn2 cores.

Sharding: core = 4*b + r (b = batch of 2, r = rank in a 4-core group).
  - Queries: core handles row chunks [512r, 512r+512) and [512(7-r), 512(8-r))
    of its batch (pairing r with 7-r balances causal attention work).
  - K/V: core projects keys [1024r, 1024(r+1)); shards are exchanged with 4
    pipelined 8-core AllGathers (K and V per key *half*, Shared outputs ->
    RDH fast path: ~40us per 1MB-in AG vs ~26us per 512KB-in 4-core mesh
    AG). Each core consumes only its batch's half of each AG output via a
    dynamic-offset DMA (offset = (core>=4) * half); explicit dep edges pin
    those loads behind the collective.
  - A tiny 8-core AllGather is emitted first so the ~40us NRT comm-init
    barrier + stream priming overlap the input DMAs / projections.
  - PE order: warmup, K01 proj, Q^T, V01 proj, K23 proj, V23 proj, then
    attention; Q^T is early so scores can start as soon as AG K01 lands.
Attention is computed in a transposed-scores layout (keys on PSUM partitions):
  S^T = K^T-chunk^T-matmul-Q^T, P^T = exp(S^T/32) * causal01mask,
  partial O/rowsum accumulate in PSUM per (quarter, query-group), then fold
  into SBUF fp32 accumulators; final O /= rowsum.
  No max-subtraction: scores are ~N(0,1) so exp cannot overflow fp32.
"""

import os
import sys

sys.path.insert(0, "/opt/trn_rl_repo")

import numpy as np
import ml_dtypes

B, S, D = 2, 4096, 1024
NCORES = 8
P = 128
NQ = 1024          # queries per core
QG = 256           # query group (scores matmul free dim)
NQG = NQ // QG     # 4
KB = 512
QK = 256           # quarter-of-rank key block
BF16 = ml_dtypes.bfloat16

# queries are stride-4 interleaved: core 4b+r handles batch-b positions
# {4i + r}. qg g (local i in [256g, 256g+256)) spans positions
# [1024g, 1024(g+1)) -> needs key ranks 0..g only.
RRS = ((0,), (0, 1), (0, 1, 2), (0, 1, 2, 3))
# score tiles per query-group PAIR: (rank, column offset, width); ranks
# needed by only the pair's second qg get a half-width tile
SCORE_TILES = (
    ((0, 0, 512), (1, 256, 256)),
    ((0, 0, 512), (1, 0, 512), (2, 0, 512), (3, 256, 256)),
)
NSTEP = 4 * sum(len(t) for t in SCORE_TILES)  # 24 mask tiles of [P, 2, 512]

LAST_EXEC_NS = None
WARMUP = int(os.environ.get("KWARMUP", "24"))

_built = {}


def _build():
    import concourse.bacc as bacc
    import concourse.tile as tile
    import concourse.mybir as mybir
    from concourse.bass_types import AP
    from concourse.tile import add_dep_helper
    from concourse.masks import make_identity

    nc = bacc.Bacc("TRN2", target_bir_lowering=False, debug=False,
                   num_devices=NCORES)
    dt = mybir.dt

    xq_t = nc.dram_tensor("xq_t", [D, NQ], dt.bfloat16, kind="ExternalInput").ap()
    xkv_t = nc.dram_tensor("xkv_t", [D, 1024], dt.bfloat16, kind="ExternalInput").ap()
    w = nc.dram_tensor("w", [D, 3 * D], dt.bfloat16, kind="ExternalInput").ap()
    maskt = nc.dram_tensor("maskt", [NSTEP, P, 2, 2 * QG], dt.bfloat16,
                           kind="ExternalInput").ap()
    out = nc.dram_tensor("out", [NQ, D], dt.float32, kind="ExternalOutput").ap()

    DC = D // P  # 8 contraction chunks
    RG8 = [[0, 1, 2, 3, 4, 5, 6, 7]]

    with tile.TileContext(nc, num_cores=NCORES) as tc:
        with (
            tc.tile_pool(name="persist", bufs=1) as persist,
            tc.tile_pool(name="dram", bufs=1, space="DRAM") as dram,
        ):
            qt_sb = persist.tile([P, DC, NQ], dt.bfloat16)
            ones_sb = persist.tile([P, P], dt.bfloat16)
            ident = persist.tile([P, P], dt.float32)

            # ---- tiny 8-core AG first: absorbs the NRT comm-init barrier
            # + CC stream priming under the input DMAs / projections.
            dum_sb = persist.tile([1, 16], dt.bfloat16)
            nc.vector.memset(dum_sb, 0.0)
            dum_in = dram.tile([1, 16], dt.bfloat16)
            dum_out = dram.tile([8, 16], dt.bfloat16)
            nc.scalar.dma_start(dum_in, dum_sb)
            nc.gpsimd.collective_compute(
                "AllGather", mybir.AluOpType.bypass, replica_groups=RG8,
                ins=[dum_in.opt()], outs=[dum_out.opt()])

            nc.vector.memset(ones_sb, 1.0)
            make_identity(nc, ident)

            # K halves: [1024 d-rows, 512 keys] per core -> out [8192, 512]
            agin_k = [dram.tile([D, 2 * QK], dt.bfloat16, name=f"agin_k{h}")
                      for h in range(2)]
            agout_k = [dram.tile([8 * D, 2 * QK], dt.bfloat16,
                                 name=f"agout_k{h}", addr_space="Shared")
                       for h in range(2)]
            # V halves: [512 keys, 1024 d] per core -> out [4096, 1024]
            agin_v = [dram.tile([2 * QK, D], dt.bfloat16, name=f"agin_v{h}")
                      for h in range(2)]
            agout_v = [dram.tile([8 * 2 * QK, D], dt.bfloat16,
                                 name=f"agout_v{h}", addr_space="Shared")
                       for h in range(2)]
            cc_k = [None, None]
            cc_v = [None, None]

            # ---- Phase 1: projections + 4 pipelined 8-core AllGathers ----
            with (
                tc.tile_pool(name="projbuf", bufs=1) as projbuf,
                tc.tile_pool(name="projtmp", bufs=4) as projtmp,
                tc.tile_pool(name="projps", bufs=4, space="PSUM") as projps,
            ):
                # PE warmup while input DMAs stream
                if WARMUP:
                    wu = projbuf.tile([P, KB], dt.bfloat16)
                    nc.vector.memset(wu, 0.0)
                    wu_ps = projps.tile([P, KB], dt.float32, tag="pps",
                                        name="wu_ps")
                    for i in range(WARMUP):
                        nc.tensor.matmul(wu_ps, lhsT=wu[:, :P], rhs=wu,
                                         start=True, stop=True)

                # alloc order matters: xq/w_q are released right after Q^T
                # (~80us); placing them last puts them at the SBUF boundary
                # where phase-2 K/V tiles overlap, so those loads unblock as
                # soon as Q^T is done instead of after all projections
                w_k = projbuf.tile([P, DC, D], dt.bfloat16)
                w_v = projbuf.tile([P, DC, D], dt.bfloat16)
                xkv_sb = projbuf.tile([P, DC, 1024], dt.bfloat16)
                xq_sb = projbuf.tile([P, DC, NQ], dt.bfloat16)
                w_q = projbuf.tile([P, DC, D], dt.bfloat16)
                w_r = w.rearrange("(c p) n -> p c n", p=P)
                # split across the two HWDGE queues so K-proj inputs land
                # first; keep gpsimd free of everything but the collective
                # triggers (smaller ucode library -> earlier first trigger)
                nc.sync.dma_start(xkv_sb, xkv_t.rearrange("(c p) n -> p c n", p=P))
                nc.scalar.dma_start(w_k, w_r[:, :, D:2 * D])
                nc.sync.dma_start(w_q, w_r[:, :, 0:D])
                nc.scalar.dma_start(xq_sb, xq_t.rearrange("(c p) n -> p c n", p=P))
                nc.sync.dma_start(w_v, w_r[:, :, 2 * D:3 * D])

                def proj_k_half(h):
                    agin_k_r = agin_k[h].rearrange("(m p) k -> m p k", p=P)
                    for m in range(DC):
                        kt_ps = projps.tile([P, 2 * QK], dt.float32,
                                            tag="ppsk", name="kt_ps")
                        for c in range(DC):
                            nc.tensor.matmul(
                                kt_ps,
                                lhsT=w_k[:, c, m * P:(m + 1) * P],
                                rhs=xkv_sb[:, c, h * 2 * QK:(h + 1) * 2 * QK],
                                start=(c == 0), stop=(c == DC - 1),
                            )
                        kt_bf = projtmp.tile([P, 2 * QK], dt.bfloat16,
                                             tag="pck")
                        nc.vector.tensor_copy(kt_bf, kt_ps)
                        nc.scalar.dma_start(agin_k_r[m], kt_bf)

                def ag_k_half(h):
                    cc_k[h] = nc.gpsimd.collective_compute(
                        "AllGather", mybir.AluOpType.bypass, replica_groups=RG8,
                        ins=[agin_k[h].opt()], outs=[agout_k[h].opt()])

                def proj_v_quarter(q):
                    h, qh = divmod(q, 2)
                    agin_v_r = agin_v[h].rearrange("(m p) d -> m p d", p=P)
                    for m in range(2):
                        for nh in range(2):
                            v_ps = projps.tile([P, KB], dt.float32, tag="pps",
                                               name="v_ps")
                            for c in range(DC):
                                nc.tensor.matmul(
                                    v_ps,
                                    lhsT=xkv_sb[:, c,
                                                q * QK + m * P:
                                                q * QK + (m + 1) * P],
                                    rhs=w_v[:, c, nh * KB:(nh + 1) * KB],
                                    start=(c == 0), stop=(c == DC - 1),
                                )
                            v_bf = projtmp.tile([P, KB], dt.bfloat16,
                                                tag="pcopy")
                            nc.vector.tensor_copy(v_bf, v_ps)
                            nc.scalar.dma_start(
                                agin_v_r[qh * 2 + m][:, nh * KB:(nh + 1) * KB],
                                v_bf)

                def ag_v_half(h):
                    cc_v[h] = nc.gpsimd.collective_compute(
                        "AllGather", mybir.AluOpType.bypass, replica_groups=RG8,
                        ins=[agin_v[h].opt()], outs=[agout_v[h].opt()])

                def proj_q():
                    for m in range(DC):
                        for nh in range(2):
                            q_ps = projps.tile([P, KB], dt.float32, tag="pps",
                                               name="q_ps")
                            for c in range(DC):
                                nc.tensor.matmul(
                                    q_ps,
                                    lhsT=w_q[:, c, m * P:(m + 1) * P],
                                    rhs=xq_sb[:, c, nh * KB:(nh + 1) * KB],
                                    start=(c == 0), stop=(c == DC - 1),
                                )
                            nc.vector.tensor_copy(
                                qt_sb[:, m, nh * KB:(nh + 1) * KB], q_ps)

                # PE order: K01 proj -> AG; Q^T (under AG); V01; K23; V23.
                proj_k_half(0)
                ag_k_half(0)
                proj_q()
                proj_v_quarter(0)
                proj_v_quarter(1)
                ag_v_half(0)
                proj_k_half(1)
                ag_k_half(1)
                proj_v_quarter(2)
                proj_v_quarter(3)
                ag_v_half(1)

            # ---- Phase 2: attention, quarter-major ----
            _phase2(nc, tc, mybir, AP, add_dep_helper, qt_sb, ones_sb, ident,
                    agout_k, agout_v, cc_k, cc_v, maskt, out)

    nc.compile()
    return nc


def _dyn_batch_ap(AP, base_ap, sel, half_elems):
    """base_ap shifted by sel*half_elems elements (sel = 0/1 runtime value)."""
    return AP(
        tensor=base_ap.tensor,
        offset=base_ap.offset + sel * half_elems,
        ap=base_ap.ap,
        dep_tracking_offset=base_ap.offset,
    )


def _phase2(nc, tc, mybir, AP, add_dep_helper, qt_sb, ones_sb, ident,
            agout_k, agout_v, cc_k, cc_v, maskt, out):
    dt = mybir.dt
    DC = D // P
    # batch selector (0 for cores 0-3, 1 for cores 4-7) per DMA engine
    sel_sync = nc.sync.partition_id() >= 4
    sel_scal = nc.scalar.partition_id() >= 4
    k_half_elems = 4 * D * 2 * QK      # 4096 rows x 512 cols
    v_half_elems = 4 * 2 * QK * D      # 2048 rows x 1024 cols
    with (
        tc.tile_pool(name="acc", bufs=1) as accpool,
        tc.tile_pool(name="kvq", bufs=1) as kvqpool,
        tc.tile_pool(name="mask", bufs=8) as maskpool,
        tc.tile_pool(name="pt", bufs=3) as ptpool,
        tc.tile_pool(name="norm", bufs=2) as normpool,
        tc.tile_pool(name="osb", bufs=2) as osbpool,
        tc.tile_pool(name="ops", bufs=5, space="PSUM") as opspool,
        tc.tile_pool(name="stps", bufs=2, space="PSUM") as stpspool,
        tc.tile_pool(name="sumps", bufs=1, space="PSUM") as sumpspool,
    ):
        o_acc = [[accpool.tile([P, D], dt.float32, name=f"oacc{qg}_{qs}")
                  for qs in range(2)] for qg in range(NQG)]
        sum_acc = [accpool.tile([P, 2 * QG], dt.float32, name=f"sacc{qgp}")
                   for qgp in range(2)]

        kall = {}      # h -> K^T tile [P, 32, 512]  (index rr*DC + c)
        vall = {}      # h -> V tile [P, 16, 1024]   (index rr*4 + qh*2 + kt)
        pt_tiles = {}  # (q, qgp, rr, kt) -> P^T tile
        state = {"step": 0}

        def load_k(h):
            kt_t = kvqpool.tile([P, 4 * DC, 2 * QK], dt.bfloat16,
                                tag="kall", name=f"kall{h}")
            src = agout_k[h][0:4 * D].rearrange("(a p) k -> p a k", p=P)
            # two parallel 2MB loads (ranks 0-1 on sync, 2-3 on scalar):
            # scores' first tiles need rank 0/1 soonest
            ld = nc.sync.dma_start(
                kt_t[:, 0:2 * DC],
                _dyn_batch_ap(AP, src[:, 0:2 * DC], sel_sync, k_half_elems))
            add_dep_helper(ld.ins, cc_k[h].ins,
                           reason="k load waits on AG completion")
            ld = nc.scalar.dma_start(
                kt_t[:, 2 * DC:4 * DC],
                _dyn_batch_ap(AP, src[:, 2 * DC:4 * DC], sel_scal,
                              k_half_elems))
            add_dep_helper(ld.ins, cc_k[h].ins,
                           reason="k load waits on AG completion")
            kall[h] = kt_t

        def load_v(h):
            v_t = kvqpool.tile([P, 16, D], dt.bfloat16, tag="vall",
                               name=f"vall{h}")
            src = agout_v[h][0:8 * QK].rearrange("(a p) d -> p a d", p=P)
            ld = nc.sync.dma_start(
                v_t[:, 0:8],
                _dyn_batch_ap(AP, src[:, 0:8], sel_sync, v_half_elems))
            add_dep_helper(ld.ins, cc_v[h].ins,
                           reason="v load waits on AG completion")
            ld = nc.scalar.dma_start(
                v_t[:, 8:16],
                _dyn_batch_ap(AP, src[:, 8:16], sel_scal, v_half_elems))
            add_dep_helper(ld.ins, cc_v[h].ins,
                           reason="v load waits on AG completion")
            vall[h] = v_t

        def pass_scores(q):
            h, qh = divmod(q, 2)
            for qgp in range(2):
                qoff = qgp * 2 * QG
                tiles = SCORE_TILES[qgp]
                sum_ps = sumpspool.tile([P, 2 * QG], dt.float32,
                                        tag="sum_ps")
                for rr, off, wd in tiles:
                    mask_sb = maskpool.tile([P, 2, 2 * QG], dt.bfloat16,
                                            tag="mask")
                    nc.sync.dma_start(mask_sb, maskt[state["step"]])
                    for kt in range(2):
                        st_ps = stpspool.tile([P, 2 * QG], dt.float32,
                                              tag="st")
                        koff = qh * QK + kt * P
                        for c in range(DC):
                            nc.tensor.matmul(
                                st_ps[:, 0:wd],
                                lhsT=kall[h][:, rr * DC + c, koff:koff + P],
                                rhs=qt_sb[:, c, qoff + off:qoff + off + wd],
                                start=(c == 0), stop=(c == DC - 1),
                            )
                        pt_sb = ptpool.tile([P, 2 * QG], dt.bfloat16,
                                            tag="pt", bufs=26,
                                            name=f"pt{q}_{qgp}_{rr}_{kt}")
                        nc.scalar.activation(
                            out=pt_sb[:, 0:wd], in_=st_ps[:, 0:wd],
                            func=mybir.ActivationFunctionType.Exp,
                            scale=float(1.0 / np.sqrt(D)),
                        )
                        nc.vector.tensor_mul(
                            pt_sb[:, 0:wd], pt_sb[:, 0:wd],
                            mask_sb[:, kt, off:off + wd])
                        pt_tiles[(q, qgp, rr, kt)] = (pt_sb, off, wd)
                    state["step"] += 1
                # rowsums after the pair's score tiles: no PE bubble waiting
                # for each tile's exp+mask to drain
                for i, (rr, off, wd) in enumerate(tiles):
                    for kt in range(2):
                        nc.tensor.matmul(
                            sum_ps[:, off:off + wd], lhsT=ones_sb,
                            rhs=pt_tiles[(q, qgp, rr, kt)][0][:, 0:wd],
                            start=(i == 0 and kt == 0),
                            stop=(i == len(tiles) - 1 and kt == 1),
                        )
                # fold the pair's rowsum into the SBUF accumulator
                if q == 0:
                    nc.vector.tensor_copy(sum_acc[qgp], sum_ps)
                else:
                    nc.vector.tensor_add(sum_acc[qgp], sum_acc[qgp], sum_ps)

        def pass_pv_half(h):
            # both quarters of half h accumulate into one PSUM group per qg:
            # half as many PSUM->SBUF folds
            for qg in range(NQG):
                qgp, qglo = divmod(qg, 2)
                rrs = RRS[qg]
                po = qglo * QG
                # four 1-bank partial-O tiles (qs, dn) with 5 slots so the
                # next qg can start accumulating while folds drain
                o_ps = [opspool.tile([P, KB], dt.float32, tag="opart", bufs=5,
                                     name=f"o_{h}_{qg}_{i}")
                        for i in range(4)]
                for q in (2 * h, 2 * h + 1):
                    qh = q % 2
                    for rr in rrs:
                        for kt in range(2):
                            ent = (pt_tiles.pop((q, qgp, rr, kt))
                                   if qglo == 1
                                   else pt_tiles[(q, qgp, rr, kt)])
                            pt_sb, off, wd = ent
                            mm_start = qh == 0 and rr == rrs[0] and kt == 0
                            mm_stop = qh == 1 and rr == rrs[-1] and kt == 1
                            for qs in range(2):
                                for dn in range(2):
                                    nc.tensor.matmul(
                                        o_ps[qs * 2 + dn],
                                        lhsT=pt_sb[:, po - off + qs * P:
                                                   po - off + (qs + 1) * P],
                                        rhs=vall[h][:, rr * 4 + qh * 2 + kt,
                                                    dn * KB:(dn + 1) * KB],
                                        start=mm_start, stop=mm_stop,
                                    )
                # fold partials into SBUF accumulators
                for qs in range(2):
                    for dn in range(2):
                        dst = o_acc[qg][qs][:, dn * KB:(dn + 1) * KB]
                        if h == 0:
                            nc.vector.tensor_copy(dst, o_ps[qs * 2 + dn])
                        else:
                            nc.vector.tensor_add(dst, dst, o_ps[qs * 2 + dn])

        # emission order matches the AG wire order K01 V01 K23 V23
        load_k(0)
        pass_scores(0)
        pass_scores(1)
        load_v(0)
        pass_pv_half(0)
        load_k(1)
        pass_scores(2)
        pass_scores(3)
        load_v(1)
        pass_pv_half(1)

        assert state["step"] == NSTEP

        # ---- normalize: O /= rowsum ----
        for qg in range(NQG):
            qoff = qg * QG
            qgp, qglo = divmod(qg, 2)
            for qs in range(2):
                o_sb = osbpool.tile([P, D], dt.float32, tag="o_sb")
                sumt_ps = stpspool.tile([P, P], dt.float32, tag="st")
                nc.tensor.transpose(
                    sumt_ps,
                    sum_acc[qgp][:, qglo * QG + qs * P:
                                 qglo * QG + (qs + 1) * P], ident)
                recip = normpool.tile([P, 1], dt.float32, tag="recip")
                nc.vector.reciprocal(recip, sumt_ps[:, 0:1])
                nc.vector.tensor_scalar_mul(o_sb, o_acc[qg][qs], recip)
                nc.scalar.dma_start(
                    out[qoff + qs * P:qoff + (qs + 1) * P, :], o_sb)


def _get_nc():
    if "nc" not in _built:
        _built["nc"] = _build()
    return _built["nc"]


def _host_inputs(x, W):
    """Build the 8 per-core input maps from the full inputs."""
    x = np.asarray(x)
    W = np.asarray(W)
    w_bf = W.astype(BF16)

    in_maps = []
    for core in range(NCORES):
        b, r = divmod(core, 4)
        xq = x[b, r::4]                                            # [1024, D]
        xkv = x[b, 1024 * r:1024 * (r + 1)]                        # [1024, D]
        in_maps.append({
            "xq_t": np.ascontiguousarray(xq.T).astype(BF16),
            "xkv_t": np.ascontiguousarray(xkv.T).astype(BF16),
            "w": w_bf,
            "maskt": _masks_for_rank(r),
        })
    return in_maps


_mask_cache = {}


def _masks_for_rank(r):
    if r in _mask_cache:
        return _mask_cache[r]
    qpos = 4 * np.arange(NQ, dtype=np.int64) + r
    m = np.zeros((NSTEP, P, 2, 2 * QG), dtype=BF16)
    step = 0
    for q in range(4):
        for qgp in range(2):
            qp = qpos[qgp * 2 * QG:(qgp + 1) * 2 * QG]
            for rr, off, wd in SCORE_TILES[qgp]:
                for kt in range(2):
                    kpos = 1024 * rr + QK * q + kt * P + np.arange(P)
                    m[step, :, kt, :] = (
                        kpos[:, None] <= qp[None, :]).astype(BF16)
                step += 1
    assert step == NSTEP
    _mask_cache[r] = m
    return m


def _gather(results):
    out = np.empty((B, S, D), dtype=np.float32)
    for core in range(NCORES):
        b, r = divmod(core, 4)
        out[b, r::4] = results[core]["out"]
    return out


def kernel(x, W):
    global LAST_EXEC_NS
    from concourse import bass_utils

    nc = _get_nc()
    in_maps = _host_inputs(x, W)
    trace = os.environ.get("BASS_KERNEL_TRACE", "0") == "1"
    if trace:
        try:
            import types
            import antenv
            if "antenv.axon_hooks" not in sys.modules:
                mod = types.ModuleType("antenv.axon_hooks")
                _hook = [None]
                mod.set_axon_ntff_profile_hook = (
                    lambda fn: _hook.__setitem__(0, fn))
                mod.get_axon_ntff_profile_hook = lambda: _hook[0]
                sys.modules["antenv.axon_hooks"] = mod
                antenv.axon_hooks = mod
            from antenv.axon_hooks import (
                get_axon_ntff_profile_hook, set_axon_ntff_profile_hook)
            if get_axon_ntff_profile_hook() is None:
                from trn_agent_boot.trn_boot import _ntff_profile_via_ctypes
                set_axon_ntff_profile_hook(
                    _ntff_profile_via_ctypes("/opt/axon/libaxon_pjrt.so"))
        except Exception:
            pass
    res = bass_utils.run_bass_kernel_spmd(
        nc, in_maps, core_ids=list(range(NCORES)), trace=trace,
        tmpdir=os.environ.get("BASS_KERNEL_TRACE_DIR") or None,
    )
    LAST_EXEC_NS = res.exec_time_ns
    return _gather(res.results)
